# revision 8
# baseline (speedup 1.0000x reference)
"""Trainium2 Bass kernel for nn_CMPModel (complex density matrix).

Math (per batch b, S=128 tokens, D=256):
    R = word_emb[questions[b]]                # [S, D]
    I = cmp_emb[questions[b]] * pos[b][:, None]
    real = R^T W R + I^T W I                  # symmetric   (W = diag(weighted_q))
    imag = I^T W R - R^T W I                  # antisymmetric

We compute only C = real + imag on device. Since diag weights can migrate
to either matmul operand (they depend only on the contraction index s),
two PSUM-accumulated products with 3 prepped operand tiles:
    C = A^T r + B^T wposc
      wposc = (w*pos)*c
      A     = w*r + wposc
      B     = pos*c - r
check (per element, sum over s):
    A^T r       = R^T W R + I^T W R
    B^T wposc   = sum_s (pos*c - r)*(w*pos*c) = I^T W I - R^T W I
and recover on host during unshard (exact by symmetry):
    real = (C + C^T)/2,  imag = (C - C^T)/2.

Sharding: data-parallel over batch, 8 batches per core, embedding tables
replicated. Gather via gpsimd indirect DMA against a host-interleaved
[V, 2D] table (row q = word_emb[q] ++ cmp_emb[q]), one [P,1]-offset
gather per batch. Multi-offset indirect DMA was probed on HW and is
broken (the ucode does not read the offset values from the q2 tile at
all; source addresses come out linear in the dest run index with a
constant sub-row misalignment), so one gather instruction per batch is
a hard floor: the Pool engine's serial descriptor generation
(~994ns fixed + 0.34ns/desc per instruction) paces the pipeline at
~1.4us/batch.

Engine balance per batch (each stage must stay under the ~1.4us gather
pace): ACT does wposc (activation Copy with per-partition scale) and
half the PSUM->SBUF copies; DVE does the two scalar_tensor_tensor preps
and the other half of the copies; PE does 4 LDWEIGHTS+MATMUL pairs.
Keeping every engine under the pace (and PE gap-free so it ramps
p-state) puts the whole compute stream inside the gather window.

enable_partition_id=False drops the per-engine TENSOR_LOAD of the
partition-id word (~1.3us of serial preamble); this kernel is SPMD-
identical across cores and never reads it.
"""

import ml_dtypes
import numpy as np

import concourse.bacc as bacc
import concourse.bass as bass
import concourse.mybir as mybir
import concourse.tile as tile
from concourse.bass_utils import run_bass_kernel_spmd

V, D, S, B = 50000, 256, 128, 64
NCORES = 8
NB = B // NCORES          # batches per core
P = 128
F32 = mybir.dt.float32
# matmul input dtype: float32r (1 cyc/row @ N>=256, ~2e-4 rel err).
# bf16 was measured in a previous session at no speedup (the span is
# gather + fixed-overhead bound, PE hides under it), so keep f32r.
USE_BF16 = False
F32R = mybir.dt.bfloat16 if USE_BF16 else mybir.dt.float32r
TBL_NP = ml_dtypes.bfloat16 if USE_BF16 else np.float32
MUL = mybir.AluOpType.mult
ADD = mybir.AluOpType.add
SUB = mybir.AluOpType.subtract

# set by test harness: trace the run and stash exec_time_ns
TRACE = False
LAST_EXEC_NS = None
LAST_RESULTS = None


def build_bass():
    nc = bacc.Bacc("TRN2", enable_partition_id=False)
    tables = nc.declare_dram_parameter("tables", [V, 2 * D], F32R, isOutput=False)
    q2_d = nc.declare_dram_parameter("q2", [P, NB], mybir.dt.int32, isOutput=False)
    pos_d = nc.declare_dram_parameter("pos_t", [P, NB], F32, isOutput=False)
    wq_d = nc.declare_dram_parameter("wq", [P, 1], F32, isOutput=False)
    out_d = nc.declare_dram_parameter("outc", [NB, P, 2, D], F32, isOutput=True)

    # (A pre-barrier q2 load via raw tensor + manual sem was tried and
    # measured WORSE by ~1us: gpsimd then holds the tile entry barrier
    # until q2 lands, so the barrier cost moves after the DMA instead of
    # overlapping it. Keep q2 as the first in-context gpsimd DMA.)
    with tile.TileContext(nc) as tc:
        with (
            tc.tile_pool(name="const", bufs=1) as constp,
            tc.tile_pool(name="gather", bufs=NB) as gatherp,
            tc.tile_pool(name="work", bufs=3 * NB) as workp,
            tc.tile_pool(name="outp", bufs=8) as outp,
            tc.tile_pool(name="psum", bufs=8, space="PSUM") as psump,
        ):
            q2 = constp.tile([P, NB], mybir.dt.int32)
            pos = constp.tile([P, NB], F32)
            wq = constp.tile([P, 1], F32)
            nc.gpsimd.dma_start(out=q2[:], in_=q2_d[:])
            nc.sync.dma_start(out=pos[:], in_=pos_d[:])
            nc.sync.dma_start(out=wq[:], in_=wq_d[:])
            # wpos on ACT: ACT is otherwise idle here, and making this the
            # first activation hoists the ACT_TABLE_LOAD to the program
            # start (it otherwise lands behind the first gather's sem wait,
            # delaying every wposc).
            wpos = constp.tile([P, NB], F32)
            nc.scalar.mul(wpos[:], pos[:], wq[:, :1])

            # all gathers up front: gpsimd desc-gen is the serial resource;
            # per-batch tiles with NB bufs so the stream never stalls on slots
            rcs = []
            for b in range(NB):
                rc = gatherp.tile([P, 2 * D], F32R, tag=f"rc")
                rcs.append(rc)
                nc.gpsimd.indirect_dma_start(
                    out=rc[:],
                    out_offset=None,
                    in_=tables[:],
                    in_offset=bass.IndirectOffsetOnAxis(ap=q2[:, b : b + 1], axis=0),
                )

            # per-batch pipeline: each batch's prep only needs its own gather,
            # so compute trails the gather stream by ~1 batch
            pss = []
            wposcs = []
            for b in range(NB):
                rc = rcs[b]
                r_b = rc[:, 0:D]
                c_b = rc[:, D : 2 * D]
                wposc = workp.tile([P, D], F32R, tag="wposc")
                wposcs.append(wposc)
                a_t = workp.tile([P, D], F32R, tag="a")
                b_t = workp.tile([P, D], F32R, tag="b")
                # wposc on ACT (idle otherwise): Copy activation with
                # per-partition scale = (w*pos)_b
                nc.scalar.mul(wposc[:], c_b, wpos[:, b : b + 1])
                # B = c*pos_b - r  (no wposc dependency, issue first)
                nc.vector.scalar_tensor_tensor(
                    b_t[:], c_b, pos[:, b : b + 1], r_b, MUL, SUB
                )
                # A = r*w + wposc
                nc.vector.scalar_tensor_tensor(
                    a_t[:], r_b, wq[:, :1], wposc[:], MUL, ADD
                )

                ps = psump.tile([P, 2, D], F32, space="PSUM", tag="ps")
                pss.append(ps)
                for m in range(2):
                    msl = slice(m * P, (m + 1) * P)
                    nc.tensor.matmul(
                        ps[:, m, :], a_t[:, msl], r_b, start=True, stop=False
                    )
                    nc.tensor.matmul(
                        ps[:, m, :], b_t[:, msl], wposc[:], start=False, stop=True
                    )

            # PSUM evacuation + output. Emitted in a second loop so the
            # in-order ACT/DVE streams hold all preps first (a copy waiting
            # on matmuls can never block a later batch's prep). Even batches
            # copied on DVE, odd on ACT. Out-DMA issue is ~0.65us serial per
            # instruction on the issuing engine's sequencer, so 9 issues on
            # sync alone stretch the tail by ~6us: split them between the
            # two HWDGE engines (sync and scalar). The last batch drains in
            # two half-tiles so its DMA overlaps its copy.
            for b in range(NB):
                if b < NB - 1:
                    out_sb = outp.tile([P, 2, D], F32, tag="osb")
                    if b % 2 == 0:
                        nc.vector.tensor_copy(out_sb[:], pss[b][:])
                        nc.sync.dma_start(out=out_d[b], in_=out_sb[:])
                    else:
                        nc.scalar.copy(out_sb[:], pss[b][:])
                        nc.scalar.dma_start(out=out_d[b], in_=out_sb[:])
                else:
                    out_sb = outp.tile([P, 2, D], F32, tag="osb")
                    nc.vector.tensor_copy(out_sb[:, 0, :], pss[b][:, 0, :])
                    nc.sync.dma_start(out=out_d[b, :, 0], in_=out_sb[:, 0, :])
                    nc.scalar.copy(out_sb[:, 1, :], pss[b][:, 1, :])
                    nc.scalar.dma_start(out=out_d[b, :, 1], in_=out_sb[:, 1, :])
    nc.compile()
    return nc


_NC = None


def _get_nc():
    global _NC
    if _NC is None:
        _NC = build_bass()
    return _NC


def kernel(questions, q_position, word_emb, cmp_emb, weighted_q):
    global LAST_EXEC_NS, LAST_RESULTS
    questions = np.asarray(questions)
    q_position = np.asarray(q_position, dtype=np.float32)
    word_emb = np.asarray(word_emb, dtype=np.float32)
    cmp_emb = np.asarray(cmp_emb, dtype=np.float32)
    weighted_q = np.asarray(weighted_q, dtype=np.float32)

    # interleaved table: row q = [word_emb[q] ++ cmp_emb[q]]  -> [V, 2D]
    tables = np.ascontiguousarray(
        np.concatenate([word_emb, cmp_emb], axis=1).astype(TBL_NP)
    )
    wq = np.ascontiguousarray(weighted_q.reshape(S, 1))

    in_maps = []
    for core in range(NCORES):
        bs = slice(core * NB, (core + 1) * NB)
        in_maps.append(
            {
                "tables": tables,
                "q2": np.ascontiguousarray(questions[bs].T.astype(np.int32)),
                "pos_t": np.ascontiguousarray(q_position[bs].T),
                "wq": wq,
            }
        )

    nc = _get_nc()
    res = run_bass_kernel_spmd(nc, in_maps, list(range(NCORES)), trace=TRACE)
    LAST_EXEC_NS = res.exec_time_ns
    LAST_RESULTS = res

    # [NCORES, NB, P, 2, D] -> C [B, 256, 256] with row d = m*128 + p
    outc = np.stack([res.results[c]["outc"] for c in range(NCORES)], axis=0)
    c_all = (
        outc.reshape(B, P, 2, D).transpose(0, 2, 1, 3).reshape(B, 2 * P, D)
    )
    ct = c_all.transpose(0, 2, 1)
    real = ((c_all + ct) * 0.5).astype(np.float32)
    imag = ((c_all - ct) * 0.5).astype(np.float32)
    return real, imag
